# revision 66
# baseline (speedup 1.0000x reference)
"""Trainium2 Bass kernel for nn_AttentionSeparateQKV (B=16, N=1024, D=768, H=12).

Data-parallel over batch: 8 NeuronCores x 2 batches each. Per core:
  x arrives host-pretransposed (feature-major xT); weights host-pretransposed
  per feature-tile f (= head pair 2f, 2f+1), software-pipelined:
    qT/kT projections (fp32r matmuls, bias fused in PSUM->SBUF copy)
    v slice via XBAR DMA transpose of bf16 k
    scores[q,k] = qT_h^T @ kT_h  (head-pair row-packed, K=64, interleaved)
    exp on ScalarE (scale fused, accum_out -> softmax denominators), probs bf16
    probs -> probsT via XBAR DMA transpose (bf16)
    AV: outT pair = v^T-form matmul with probsT (head-pair col-packed)
  normalize via selector-matmul-replicated 1/denom; out-proj fp32r + bias matmul
"""

import sys

if "/opt/trn_rl_repo" not in sys.path:
    sys.path.insert(0, "/opt/trn_rl_repo")

import numpy as np

B, N, D, H = 16, 1024, 768, 12
HD = D // H                # 64
SCALE = float(HD) ** -0.5  # 0.125
N_CORES = 8
BL = B // N_CORES          # 2 batches per core
T = BL * N                 # 2048 tokens per core
FT = D // 128              # 6 feature tiles == head pairs
NQT = N // 128             # 8 query tiles per batch
QC = 512                   # query chunk (attention granularity)
NCH = N // QC              # 2 chunks per batch
QTC = QC // 128            # 4 query tiles per chunk

_NC_CACHE = []


def _build():
    import concourse.mybir as mybir
    import concourse.tile as tile
    from concourse import bacc

    F32 = mybir.dt.float32
    F32R = mybir.dt.float32r
    BF16 = mybir.dt.bfloat16
    EXP = mybir.ActivationFunctionType.Exp
    MULT = mybir.AluOpType.mult

    nc = bacc.Bacc("TRN2", target_bir_lowering=False, debug=False)

    x_d = nc.dram_tensor("x", [D, T], F32R, kind="ExternalInput").ap()  # host-pretransposed
    wq_d = nc.dram_tensor("wqt", [D, D], F32R, kind="ExternalInput").ap()
    wk_d = nc.dram_tensor("wkt", [D, D], F32R, kind="ExternalInput").ap()
    wp_d = nc.dram_tensor("wpt", [D, D], F32R, kind="ExternalInput").ap()
    bq_d = nc.dram_tensor("bqp", [128, FT], F32, kind="ExternalInput").ap()
    bk_d = nc.dram_tensor("bkp", [128, FT], F32, kind="ExternalInput").ap()
    bc_d = nc.dram_tensor("bc", [33, 1024], F32R, kind="ExternalInput").ap()
    id_d = nc.dram_tensor("ident", [128, 128], F32, kind="ExternalInput").ap()
    out_d = nc.dram_tensor("out", [T, D], F32, kind="ExternalOutput").ap()

    with tile.TileContext(nc) as tc:
        with (
            tc.tile_pool(name="const", bufs=1) as cpool,
            tc.tile_pool(name="perb", bufs=1) as bpool,
            tc.tile_pool(name="proj", bufs=3) as proj_pool,
            tc.tile_pool(name="probs", bufs=4) as probs_pool,
            tc.tile_pool(name="probsT", bufs=5) as pT_pool,
            tc.tile_pool(name="dn", bufs=2) as dn_pool,
            tc.tile_pool(name="outTp", bufs=1) as outT_pool,
            tc.tile_pool(name="fin", bufs=3) as fin_pool,
            tc.tile_pool(name="ps_big", bufs=2, space="PSUM") as ps_big,
            tc.tile_pool(name="ps_av", bufs=2, space="PSUM") as ps_av,
            tc.tile_pool(name="ps_misc", bufs=2, space="PSUM") as ps_misc,
        ):
            # ---- small constants needed immediately ----
            id_sb = cpool.tile([128, 128], F32, tag="ident")
            nc.scalar.dma_start(id_sb[:], id_d[:])
            bq_sb = cpool.tile([128, FT], F32, tag="bq")
            nc.scalar.dma_start(bq_sb[:], bq_d[:])
            bk_sb = cpool.tile([128, FT], F32, tag="bk")
            nc.scalar.dma_start(bk_sb[:], bk_d[:])

            def emit_xT(b):
                """Load feature-major x slice for batch b (host-pretransposed)."""
                xT = bpool.tile([128, FT, N], F32R, tag="xT")
                nc.gpsimd.dma_start(
                    xT[:],
                    x_d[:, b * N : (b + 1) * N].rearrange("(ko kp) t -> kp ko t", kp=128),
                )
                return xT

            def emit_proj(xT, f):
                """Q/K projections for feature tile f; returns (qTf, kTf, vf)."""
                qTf = proj_pool.tile([128, N], F32R, tag="qTf")
                kTf = proj_pool.tile([128, N], F32R, tag="kTf")
                vf = proj_pool.tile([128, NQT, 128], BF16, tag="vf")
                k16 = probs_pool.tile([128, N], BF16, tag="probs")
                for qs in range(N // 512):
                    pq = ps_misc.tile([128, 512], F32, tag="misc")
                    for ks in range(FT):
                        nc.tensor.matmul(
                            pq[:],
                            wq_sb[:, ks, 128 * f : 128 * (f + 1)],
                            xT[:, ks, 512 * qs : 512 * (qs + 1)],
                            start=(ks == 0),
                            stop=(ks == FT - 1),
                        )
                    nc.vector.tensor_scalar_add(
                        qTf[:, 512 * qs : 512 * (qs + 1)], pq[:], bq_sb[:, f : f + 1]
                    )
                    pk = ps_misc.tile([128, 512], F32, tag="misc")
                    for ks in range(FT):
                        nc.tensor.matmul(
                            pk[:],
                            wk_sb[:, ks, 128 * f : 128 * (f + 1)],
                            xT[:, ks, 512 * qs : 512 * (qs + 1)],
                            start=(ks == 0),
                            stop=(ks == FT - 1),
                        )
                    nc.vector.tensor_scalar_add(
                        kTf[:, 512 * qs : 512 * (qs + 1)], pk[:], bk_sb[:, f : f + 1]
                    )
                    nc.vector.tensor_scalar_add(
                        k16[:, 512 * qs : 512 * (qs + 1)], pk[:], bk_sb[:, f : f + 1]
                    )
                nc.sync.dma_start(vf[:], k16[:], transpose=True)
                return qTf, kTf, vf

            def emit_normalize(outT, denom, c):
                """Chunk-c softmax normalization of outT via replicated 1/denom."""
                recip = dn_pool.tile([128, QTC * H], F32, tag="recip")
                nc.vector.reciprocal(recip[:], denom[:])
                recipT = dn_pool.tile([H, QTC, 128], F32R, tag="recipT")
                for ql in range(QTC):
                    pt = ps_misc.tile([128, 512], F32, tag="misc")
                    nc.tensor.transpose(
                        pt[0:H, 0:128], recip[:, ql * H : (ql + 1) * H], id_sb[:]
                    )
                    nc.vector.tensor_copy(recipT[:, ql, :], pt[0:H, 0:128])
                for f in range(FT):
                    rp = ps_misc.tile([128, 512], F32, tag="misc")
                    nc.tensor.matmul(
                        rp[:], bc_sb[0:H, 128 * f : 128 * (f + 1)], recipT[:], start=True, stop=True
                    )
                    nc.vector.tensor_tensor(
                        outT[:, f, QC * c : QC * (c + 1)],
                        outT[:, f, QC * c : QC * (c + 1)],
                        rp[:],
                        MULT,
                    )

            def emit_outproj(outT, b, tts):
                """Final projection + bias for token tiles `tts` of batch b."""
                tok0 = b * N
                for tt in tts:
                    fin = fin_pool.tile([128, D], F32, tag="fin")
                    for ns in range(2):
                        pf = ps_misc.tile([128, 512], F32, tag="misc")
                        for ks in range(FT):
                            nc.tensor.matmul(
                                pf[:, 0:384],
                                outT[:, ks, 128 * tt : 128 * (tt + 1)],
                                wp_sb[:, ks, 384 * ns : 384 * (ns + 1)],
                                start=(ks == 0),
                                stop=False,
                            )
                        nc.tensor.matmul(
                            pf[:, 0:384],
                            bc_sb[32:33, 768:896],
                            bc_sb[32:33, 384 * ns : 384 * (ns + 1)],
                            start=False,
                            stop=True,
                        )
                        nc.vector.tensor_copy(fin[:, 384 * ns : 384 * (ns + 1)], pf[:, 0:384])
                    nc.gpsimd.dma_start(
                        out_d[tok0 + 128 * tt : tok0 + 128 * (tt + 1), :], fin[:]
                    )

            pending = []  # deferred out-projection pieces
            xT = emit_xT(0)
            wq_sb = cpool.tile([128, FT, D], F32R, tag="wq")
            wk_sb = cpool.tile([128, FT, D], F32R, tag="wk")
            wq_r = wq_d.rearrange("(ko kp) m -> kp ko m", kp=128)
            wk_r = wk_d.rearrange("(ko kp) m -> kp ko m", kp=128)
            for wf in range(FT):
                sl = slice(128 * wf, 128 * (wf + 1))
                nc.scalar.dma_start(wq_sb[:, :, sl], wq_r[:, :, sl])
                nc.scalar.dma_start(wk_sb[:, :, sl], wk_r[:, :, sl])
            nxt = emit_proj(xT, 0)
            wp_sb = cpool.tile([128, FT, D], F32R, tag="wp")
            nc.scalar.dma_start(wp_sb[:], wp_d.rearrange("(ko kp) m -> kp ko m", kp=128))
            bc_sb = cpool.tile([33, 1024], F32R, tag="bc")
            nc.scalar.dma_start(bc_sb[:], bc_d[:])
            xT_next = None
            pend_av = None

            def flush_av(outT, vf, pT_tiles, denoms, f, c, b):
                """Emit the deferred AV matmuls + outT copy for stage (b,f,c),
                plus per-chunk normalization and out-proj scheduling."""
                av = ps_av.tile([128, QC], F32, tag="av")
                for ks in range(NQT):
                    for e in range(2):
                        nc.tensor.matmul(
                            av[64 * e : 64 * (e + 1), :],
                            vf[:, ks, 64 * e : 64 * (e + 1)],
                            pT_tiles[e][:, ks, :],
                            start=(ks == 0),
                            stop=(ks == NQT - 1),
                        )
                nc.vector.tensor_copy(outT[:, f, QC * c : QC * (c + 1)], av[:])
                if f == FT - 1:
                    emit_normalize(outT, denoms[c], c)
                    pending.extend(
                        (outT, b, range(c * QTC + 2 * i, c * QTC + 2 * i + 2))
                        for i in range(QTC // 2)
                    )

            for b in range(BL):
                outT = outT_pool.tile([128, FT, N], F32R, tag="outT")
                denoms = [
                    dn_pool.tile([128, QTC * H], F32, tag="denom", name="denom")
                    for _ in range(NCH)
                ]
                for f in range(FT):
                    qTf, kTf, vf = nxt
                    for c in range(NCH):
                        denom = denoms[c]
                        pT_tiles = {}
                        for e in range(2):
                            pT_tiles[e] = pT_pool.tile(
                                [128, NQT, QC], BF16, tag="probsT", name="pT"
                            )
                        for ql in range(QTC):
                            qt = c * QTC + ql
                            sc = {}
                            for e in range(2):
                                sc[e] = ps_big.tile([128, N], F32, tag="sc", name="sc")
                            for kt in range(N // 512):
                                for e in range(2):
                                    nc.tensor.matmul(
                                        sc[e][:, 512 * kt : 512 * (kt + 1)],
                                        qTf[64 * e : 64 * (e + 1), 128 * qt : 128 * (qt + 1)],
                                        kTf[64 * e : 64 * (e + 1), 512 * kt : 512 * (kt + 1)],
                                        start=True,
                                        stop=True,
                                    )
                            for e in range(2):
                                h = 2 * f + e
                                pr = probs_pool.tile([128, N], BF16, tag="probs")
                                nc.scalar.activation(
                                    pr[:], sc[e][:], EXP, scale=SCALE,
                                    accum_out=denom[:, ql * H + h : ql * H + h + 1],
                                )
                                nc.sync.dma_start(
                                    pT_tiles[e][:, :, 128 * ql : 128 * (ql + 1)],
                                    pr[:],
                                    transpose=True,
                                )
                        # PE filler work emitted while ACT/DMA chew exp+transpose:
                        # next projections, the next batch's xT load, and the
                        # previous rounds' out-projection in 2-token-tile pieces.
                        if c == 0:
                            if f + 1 < FT:
                                nxt = emit_proj(xT, f + 1)
                        else:
                            if f == FT - 2 and b + 1 < BL:
                                xT_next = emit_xT(b + 1)
                            if f == FT - 1 and b + 1 < BL:
                                xT = xT_next
                                nxt = emit_proj(xT, 0)
                        # AV of the PREVIOUS stage (deferred so this stage's
                        # scores/exp issue before PE blocks on the previous
                        # stage's probs transposes)
                        if pend_av is not None:
                            flush_av(*pend_av)
                        for _ in range(2):
                            if pending:
                                emit_outproj(*pending.pop(0))
                        pend_av = (outT, vf, pT_tiles, denoms, f, c, b)
            flush_av(*pend_av)
            while pending:
                emit_outproj(*pending.pop(0))

    nc.compile()
    return nc


def _get_nc():
    if not _NC_CACHE:
        _NC_CACHE.append(_build())
    return _NC_CACHE[0]


def _to_np(a):
    try:
        return np.asarray(a)
    except Exception:
        import jax

        return np.asarray(jax.device_get(a))


def _prep_inputs(x, Wq, bq, Wk, bk, Wp, bp):
    x, Wq, bq, Wk, bk, Wp, bp = (
        _to_np(a) for a in (x, Wq, bq, Wk, bk, Wp, bp)
    )
    x = np.ascontiguousarray(np.asarray(x, dtype=np.float32))
    wqt = np.ascontiguousarray(np.asarray(Wq, np.float32).T)
    wkt = np.ascontiguousarray(np.asarray(Wk, np.float32).T)
    wpt = np.ascontiguousarray(np.asarray(Wp, np.float32).T)
    bqp = np.ascontiguousarray(np.asarray(bq, np.float32).reshape(FT, 128).T)
    bkp = np.ascontiguousarray(np.asarray(bk, np.float32).reshape(FT, 128).T)
    bc = np.zeros((33, 1024), np.float32)
    cols = np.arange(D)
    bc[cols // HD, cols] = 1.0                      # selector rows 0..11
    bc[32, :D] = np.asarray(bp, np.float32)         # output bias row (base 32)
    bc[32, D : D + 128] = 1.0                       # ones segment for bias matmul
    ident = np.eye(128, dtype=np.float32)
    base = {
        "wqt": wqt, "wkt": wkt, "wpt": wpt,
        "bqp": bqp, "bkp": bkp, "bc": bc, "ident": ident,
    }
    in_maps = []
    for c in range(N_CORES):
        xc = np.ascontiguousarray(x[BL * c : BL * (c + 1)].reshape(T, D).T)
        in_maps.append(dict(base, x=xc))
    return in_maps


def _run(in_maps, **kw):
    from concourse.bass_utils import run_bass_kernel_spmd

    nc = _get_nc()
    return run_bass_kernel_spmd(nc, in_maps, core_ids=list(range(N_CORES)), **kw)


def kernel(x, Wq, bq, Wk, bk, Wp, bp):
    in_maps = _prep_inputs(x, Wq, bq, Wk, bk, Wp, bp)
    res = _run(in_maps)
    out = np.concatenate(
        [r["out"].reshape(BL, N, D) for r in res.results], axis=0
    )
    return out.astype(np.float32)
